# revision 1
# baseline (speedup 1.0000x reference)
"""Batched sparse forward projection Y[b,r] = sum_k vals[k]*X[b,cols[k]] for rows[k]==r.

Strategy (8 NeuronCores, row-sharded):
- Each core owns a 16384-row slice; nnz slice via searchsorted (rows sorted).
- nnz bucketed by col>>13 into 8 buckets = 8 GPSIMD Q7 cores; stable bucketing
  keeps rows sorted per bucket.
- Gather via ap_gather: X table [128ch, 8192, 1] f32, channel 16c+j (j<8) holds
  X[j, 8192c+e]; per-Q7-core wrapped int16 index lists fetch all 8 batch values
  per nnz.
- Per chunk (768 output rows): contrib = gathered * vals (DVE), then a plain
  free-dim cumsum via tensor_tensor_scan, then a second ap_gather extracts the
  cumsum at each row's last-slot position (ends list, with a leading zero-slot);
  adjacent diffs give per-row/bucket/batch totals; a [128,8] selection matmul
  sums buckets into PSUM [8, rows].
- Empty rows need no slots: their end position inherits the previous row's,
  so the diff is zero.
"""

import numpy as np

import concourse.bass as bass
import concourse.mybir as mybir
import concourse.tile as tile
from concourse import bacc
from concourse.bass_utils import run_bass_kernel_spmd

B = 8
N_PIX = 65536
N_ROWS = 131072
N_CORES = 8
NBUK = 8
BUK = N_PIX // NBUK  # 8192
P = 128
RPC = 768  # rows per chunk (mult of 16; empirically fastest vs 944 in A/B)

_compiled = {}


def _ceil_to(x, m):
    return -(-x // m) * m


def _prep_core(rows_l, cols_n, vals_n, rows_per_core, rpc):
    """Sort by (bucket, row); per-(bucket,chunk) slot needs (+1 zero slot)."""
    buk = (cols_n >> 13).astype(np.int64)
    e = (cols_n & (BUK - 1)).astype(np.int16)
    key = buk * rows_per_core + rows_l.astype(np.int64)
    perm = np.argsort(key, kind="stable")
    skey = key[perm]
    cnt = np.bincount(key, minlength=NBUK * rows_per_core).reshape(NBUK, rows_per_core)
    n_chunks = -(-rows_per_core // rpc)
    need = np.zeros((NBUK, n_chunks), np.int64)
    for k in range(n_chunks):
        r0, r1 = k * rpc, min((k + 1) * rpc, rows_per_core)
        need[:, k] = cnt[:, r0:r1].sum(axis=1) + 1  # +1 zero slot
    return {
        "perm": perm,
        "skey": skey,
        "cnt": cnt,
        "e": e,
        "vals": vals_n,
        "need": need,
        "n_chunks": n_chunks,
    }


def _layout_core(prep, cls_, rows_per_core, rpc):
    n_chunks = len(cls_)
    Ltot = int(sum(cls_))
    cnt = prep["cnt"]
    cbase = np.concatenate([[0], np.cumsum(cls_)]).astype(np.int64)

    skey, perm = prep["skey"], prep["perm"]
    c_sorted = skey // rows_per_core
    r_sorted = skey % rows_per_core
    chunk_id = r_sorted // rpc
    seg_key = c_sorted * n_chunks + chunk_id
    seg_cnt = np.bincount(seg_key, minlength=NBUK * n_chunks)
    seg_start = np.cumsum(seg_cnt) - seg_cnt
    rank = np.arange(skey.shape[0], dtype=np.int64) - seg_start[seg_key]
    pos = cbase[chunk_id] + 1 + rank  # +1 for the zero slot

    idx16 = np.full((NBUK, Ltot), -1, np.int16)
    valsd = np.zeros((NBUK, Ltot), np.float32)
    idx16[c_sorted, pos] = prep["e"][perm]
    valsd[c_sorted, pos] = prep["vals"][perm]

    # extraction lists per chunk: [0, ends(r0), ends(r0+1), ...] padded to rpc+16
    epl = rpc + 16
    epx = np.zeros((NBUK, n_chunks, epl), np.int16)
    ccnt = np.cumsum(cnt, axis=1)
    for k in range(n_chunks):
        r0, r1 = k * rpc, min((k + 1) * rpc, rows_per_core)
        prev = ccnt[:, r0 - 1] if r0 > 0 else np.zeros(NBUK, np.int64)
        ends = ccnt[:, r0:r1] - prev[:, None]  # last-slot pos (1-based w/ zero slot)
        epx[:, k, 1 : 1 + (r1 - r0)] = ends.astype(np.int16)
        epx[:, k, 1 + (r1 - r0) :] = ends[:, -1:].astype(np.int16)

    idxw = np.ascontiguousarray(
        idx16.reshape(NBUK, Ltot // 16, 16).transpose(0, 2, 1)
    ).reshape(NBUK * 16, Ltot // 16)
    epxw = np.ascontiguousarray(
        epx.reshape(NBUK, n_chunks * epl // 16, 16).transpose(0, 2, 1)
    ).reshape(NBUK * 16, n_chunks * epl // 16)
    return {"idxw": idxw, "valsd": valsd, "epxw": epxw}


def _build_nc(cls_, rpc, rows_per_core, repeat=1):
    n_chunks = len(cls_)
    Ltot = int(sum(cls_))
    epl = rpc + 16
    nc = bacc.Bacc("TRN2", target_bir_lowering=False, debug=False)
    f32, i16 = mybir.dt.float32, mybir.dt.int16

    xt = nc.dram_tensor("xt", [P, BUK], f32, kind="ExternalInput")
    sel = nc.dram_tensor("sel", [P, B], f32, kind="ExternalInput")
    idxw = nc.dram_tensor("idxw", [P, Ltot // 16], i16, kind="ExternalInput")
    valsd = nc.dram_tensor("valsd", [NBUK, Ltot], f32, kind="ExternalInput")
    epxw = nc.dram_tensor(
        "epxw", [P, n_chunks * epl // 16], i16, kind="ExternalInput"
    )
    y = nc.dram_tensor("y", [B, rows_per_core], f32, kind="ExternalOutput")

    CLmax = max(int(c) for c in cls_)

    with tile.TileContext(nc) as tc:
        with (
            tc.tile_pool(name="tabp", bufs=1) as tabp,
            tc.tile_pool(name="selp", bufs=1) as selp,
            tc.tile_pool(name="onep", bufs=1) as onep,
            tc.tile_pool(name="idxp", bufs=2) as idxp,
            tc.tile_pool(name="gtp", bufs=1) as gtp,
            tc.tile_pool(name="valp", bufs=1) as valp,
            tc.tile_pool(name="epp", bufs=2) as epp,
            tc.tile_pool(name="etp", bufs=2) as etp,
            tc.tile_pool(name="dtp", bufs=2) as dtp,
            tc.tile_pool(name="ysbp", bufs=2) as ysbp,
            tc.tile_pool(name="psp", bufs=2, space="PSUM") as psp,
        ):
            tab_t = tabp.tile([P, BUK, 1], f32)
            nc.sync.dma_start(tab_t[:, :, 0], xt[:])
            sel_t = selp.tile([P, B], f32)
            nc.sync.dma_start(sel_t[:], sel[:])
            ones_t = onep.tile([P, 1], f32)
            nc.vector.memset(ones_t[:], 1.0)

            for _rep in range(repeat):
                sbase = 0
                for k in range(n_chunks):
                    CL = int(cls_[k])
                    rb = k * rpc
                    rpck = min(rpc, rows_per_core - rb)

                    it = idxp.tile([P, CL // 16], i16, tag="idx")
                    nc.sync.dma_start(
                        it[:], idxw[:, sbase // 16 : (sbase + CL) // 16]
                    )
                    gt = gtp.tile([P, CL, 1], f32, tag="gt")
                    nc.gpsimd.ap_gather(
                        out_ap=gt[:],
                        in_ap=tab_t[:],
                        idxs_ap=it[:],
                        channels=P,
                        num_elems=BUK,
                        d=1,
                        num_idxs=CL,
                    )
                    vt = valp.tile([P, CL], f32, tag="val")
                    for c in range(NBUK):
                        src_v = bass.AP(valsd, c * Ltot + sbase, [[0, 16], [1, CL]])
                        nc.sync.dma_start(vt[16 * c : 16 * c + 16, :], src_v)
                    nc.vector.tensor_tensor(
                        out=gt[:, :, 0],
                        in0=gt[:, :, 0],
                        in1=vt[:],
                        op=mybir.AluOpType.mult,
                    )
                    # plain inclusive cumsum along the chunk (per partition)
                    nc.vector.tensor_tensor_scan(
                        out=gt[:, :, 0],
                        data0=ones_t[:].to_broadcast([P, CL]),
                        data1=gt[:, :, 0],
                        initial=0.0,
                        op0=mybir.AluOpType.mult,
                        op1=mybir.AluOpType.add,
                    )
                    # extract cumsum at [0, end(r0), end(r0+1), ...]
                    ep = epp.tile([P, epl // 16], i16, tag="ep")
                    nc.sync.dma_start(
                        ep[:], epxw[:, k * epl // 16 : (k + 1) * epl // 16]
                    )
                    et = etp.tile([P, epl, 1], f32, tag="et")
                    nc.gpsimd.ap_gather(
                        out_ap=et[:],
                        in_ap=gt[:],
                        idxs_ap=ep[:],
                        channels=P,
                        num_elems=CL,
                        d=1,
                        num_idxs=epl,
                    )
                    dt = dtp.tile([P, rpck], f32, tag="dt")
                    nc.vector.tensor_tensor(
                        out=dt[:],
                        in0=et[:, 1 : rpck + 1, 0],
                        in1=et[:, 0:rpck, 0],
                        op=mybir.AluOpType.subtract,
                    )
                    ps = psp.tile([B, _ceil_to(rpc, 512)], f32, tag="ps")
                    for m in range(_ceil_to(rpck, 512) // 512):
                        a, b_ = m * 512, min((m + 1) * 512, rpck)
                        nc.tensor.matmul(
                            out=ps[:, a:b_],
                            lhsT=sel_t[:],
                            rhs=dt[:, a:b_],
                            start=True,
                            stop=True,
                        )
                    ysb = ysbp.tile([B, _ceil_to(rpc, 512)], f32, tag="ysb")
                    nc.vector.tensor_copy(out=ysb[:, :rpck], in_=ps[:, :rpck])
                    nc.sync.dma_start(y[:, rb : rb + rpck], ysb[:, :rpck])
                    sbase += CL
    nc.compile()
    return nc


def _full_prep(X, vals, rows, cols, rows_per_core, rpc, n_cores):
    n_chunks = -(-rows_per_core // rpc)
    bounds = np.searchsorted(rows, np.arange(n_cores + 1) * rows_per_core)
    preps = []
    for n in range(n_cores):
        k0, k1 = bounds[n], bounds[n + 1]
        preps.append(
            _prep_core(
                (rows[k0:k1] - n * rows_per_core).astype(np.int64),
                cols[k0:k1].astype(np.int64),
                vals[k0:k1],
                rows_per_core,
                rpc,
            )
        )
    need = np.stack([p["need"] for p in preps])
    cls_ = [int(_ceil_to(int(need[:, :, k].max()), 64)) for k in range(n_chunks)]
    assert max(cls_) <= 16384, f"chunk too big: {max(cls_)}"

    T = np.zeros((P, BUK), np.float32)
    for c in range(NBUK):
        T[16 * c : 16 * c + 8, :] = X[:, BUK * c : BUK * (c + 1)]
    selm = np.zeros((P, B), np.float32)
    for c in range(NBUK):
        for j in range(B):
            selm[16 * c + j, j] = 1.0

    in_maps = []
    for n in range(n_cores):
        lay = _layout_core(preps[n], cls_, rows_per_core, rpc)
        in_maps.append(
            {
                "xt": T,
                "sel": selm,
                "idxw": lay["idxw"],
                "valsd": lay["valsd"],
                "epxw": lay["epxw"],
            }
        )
    return cls_, in_maps


def kernel(X, vals, rows, cols):
    X = np.asarray(X, np.float32)
    vals = np.asarray(vals, np.float32)
    rows = np.asarray(rows, np.int64)
    cols = np.asarray(cols, np.int64)
    rows_per_core = N_ROWS // N_CORES

    rpc = RPC
    while True:
        try:
            cls_, in_maps = _full_prep(X, vals, rows, cols, rows_per_core, rpc, N_CORES)
            break
        except AssertionError:
            rpc //= 2  # denser-than-expected chunks: halve rows per chunk
            if rpc < 64:
                raise
    key = (tuple(cls_), rpc, rows_per_core)
    if key not in _compiled:
        _compiled[key] = _build_nc(cls_, rpc, rows_per_core)
    nc = _compiled[key]
    res = run_bass_kernel_spmd(nc, in_maps, core_ids=list(range(N_CORES)))
    Y = np.concatenate([res.results[n]["y"] for n in range(N_CORES)], axis=1)
    return np.ascontiguousarray(Y, dtype=np.float32)



# revision 2
# speedup vs baseline: 8.3599x; 8.3599x over previous
"""Batched sparse projection Y[b,r] = sum_k vals[k]*X[b,cols[k]] for rows[k]==r.

Rewritten for this axon/TRN2 stack where per-core instructions execute
serially at ~25-50us fixed cost each (+ data slopes). Strategy: minimize
instruction count, maximize per-instruction work.

- 8 NeuronCores row-sharded (16384 rows each), nnz slice via searchsorted.
- Per core: nnz bucketed by col>>13 into 8 buckets = 8 GPSIMD Q7 cores.
  Table [128, 8192] f32: partition 16c+j (j<8) holds X[j, 8192c:8192(c+1)].
- Chunks of RPC=1024 rows (16 chunks). Per chunk only 7 instructions:
  1 meta DMA (idx+ends lists packed), 1 replicated vals DMA (stride-0 AP),
  1 ap_gather (num_idxs~16.5K), 1 in-place DVE mult, 1 DVE cumsum scan
  (output overwrites the vals buffer), 1 ends-extract ap_gather, 1 diff
  into f16 staging.
- Bucket merge: per 8192-row half, 3 log2 partition-fold steps
  (SBUF->SBUF shift DMA + DVE f16 add), then one f16 DMA out; host
  converts to f32.
"""

import numpy as np

import concourse.bass as bass
import concourse.mybir as mybir
import concourse.tile as tile
from concourse import bacc
from concourse.bass_utils import run_bass_kernel_spmd

B = 8
N_PIX = 65536
N_ROWS = 131072
N_CORES = 8
NBUK = 8
BUK = N_PIX // NBUK  # 8192
P = 128
RPC = 960  # rows per chunk

_compiled = {}


def _ceil_to(x, m):
    return -(-x // m) * m


def _prep_core(rows_l, cols_n, vals_n, rows_per_core, rpc):
    """Sort one core's nnz by (bucket, row); per-(bucket,chunk) counts."""
    buk = (cols_n >> 13).astype(np.int64)
    e = (cols_n & (BUK - 1)).astype(np.int16)
    key = buk * rows_per_core + rows_l.astype(np.int64)
    perm = np.argsort(key, kind="stable")
    skey = key[perm]
    cnt = np.bincount(key, minlength=NBUK * rows_per_core).reshape(NBUK, rows_per_core)
    n_chunks = -(-rows_per_core // rpc)
    need = np.zeros((NBUK, n_chunks), np.int64)
    for k in range(n_chunks):
        r0, r1 = k * rpc, min((k + 1) * rpc, rows_per_core)
        need[:, k] = cnt[:, r0:r1].sum(axis=1) + 1  # +1 zero slot
    return {"perm": perm, "skey": skey, "cnt": cnt, "e": e, "vals": vals_n,
            "need": need, "n_chunks": n_chunks}


def _layout_core(prep, S_list, epl_list, rows_per_core, rpc):
    """Build per-core meta (idx+ends wrapped int16) and valsd [8, Ltot]."""
    n_chunks = len(S_list)
    Ltot = int(sum(S_list))
    cnt = prep["cnt"]
    cbase = np.concatenate([[0], np.cumsum(S_list)]).astype(np.int64)

    skey, perm = prep["skey"], prep["perm"]
    c_sorted = skey // rows_per_core
    r_sorted = skey % rows_per_core
    chunk_id = r_sorted // rpc
    seg_key = c_sorted * n_chunks + chunk_id
    seg_cnt = np.bincount(seg_key, minlength=NBUK * n_chunks)
    seg_start = np.cumsum(seg_cnt) - seg_cnt
    rank = np.arange(skey.shape[0], dtype=np.int64) - seg_start[seg_key]
    pos = cbase[chunk_id] + 1 + rank  # +1 for the zero slot

    idx16 = np.zeros((NBUK, Ltot), np.int16)  # pad idx 0 (valid; val=0 kills it)
    valsd = np.zeros((NBUK, Ltot), np.float32)
    idx16[c_sorted, pos] = prep["e"][perm]
    valsd[c_sorted, pos] = prep["vals"][perm]

    # ends lists per chunk: [0, end(r0), end(r0+1), ...] padded with last
    ccnt = np.cumsum(cnt, axis=1)
    epx_parts = []
    for k in range(n_chunks):
        epl = epl_list[k]
        r0, r1 = k * rpc, min((k + 1) * rpc, rows_per_core)
        prev = ccnt[:, r0 - 1] if r0 > 0 else np.zeros(NBUK, np.int64)
        ends = ccnt[:, r0:r1] - prev[:, None]  # 1-based slot pos in chunk
        ep = np.zeros((NBUK, epl), np.int16)
        ep[:, 1 : 1 + (r1 - r0)] = ends.astype(np.int16)
        ep[:, 1 + (r1 - r0) :] = ends[:, -1:].astype(np.int16)
        epx_parts.append(ep)

    def wrap(a):  # [8, L] -> [128, L//16]; channel 16c+t <- a[c, 16s+t] at col s
        L = a.shape[1]
        return np.ascontiguousarray(
            a.reshape(NBUK, L // 16, 16).transpose(0, 2, 1)
        ).reshape(NBUK * 16, L // 16)

    # meta: per chunk [idxw_k | epw_k] concatenated along columns
    meta_parts = []
    for k in range(n_chunks):
        s0, s1 = int(cbase[k]), int(cbase[k + 1])
        meta_parts.append(wrap(idx16[:, s0:s1]))
        meta_parts.append(wrap(epx_parts[k]))
    meta = np.concatenate(meta_parts, axis=1)
    return {"meta": meta, "valsd": valsd}


def _build_nc(S_list, epl_list, rpc, rows_per_core, repeat=1):
    n_chunks = len(S_list)
    Ltot = int(sum(S_list))
    Mcols = int(sum(S_list[k] + epl_list[k] for k in range(n_chunks))) // 16
    nc = bacc.Bacc("TRN2", target_bir_lowering=False, debug=False)
    f32, f16, i16 = mybir.dt.float32, mybir.dt.float16, mybir.dt.int16

    xt = nc.dram_tensor("xt", [P, BUK], f32, kind="ExternalInput")
    meta = nc.dram_tensor("meta", [P, Mcols], i16, kind="ExternalInput")
    valsd = nc.dram_tensor("valsd", [NBUK, Ltot], f32, kind="ExternalInput")
    y = nc.dram_tensor("y", [B, rows_per_core], f16, kind="ExternalOutput")

    S_max = max(int(s) for s in S_list)
    epl_max = max(int(x) for x in epl_list)
    # two fold-groups split at a chunk boundary
    ksplit = -(-n_chunks // 2)
    groups = [(0, ksplit), (ksplit, n_chunks)]
    gspan = max(
        min(ke * rpc, rows_per_core) - ks * rpc for ks, ke in groups if ke > ks
    )

    with tile.TileContext(nc) as tc:
        with (
            tc.tile_pool(name="tabp", bufs=1) as tabp,
            tc.tile_pool(name="onep", bufs=1) as onep,
            tc.tile_pool(name="metap", bufs=1) as metap,
            tc.tile_pool(name="gtp", bufs=1) as gtp,
            tc.tile_pool(name="vtp", bufs=1) as vtp,
            tc.tile_pool(name="etp", bufs=1) as etp,
            tc.tile_pool(name="dtp", bufs=1) as dtp,
            tc.tile_pool(name="fp", bufs=1) as fp,
        ):
            tab_t = tabp.tile([P, BUK, 1], f32)
            nc.sync.dma_start(tab_t[:, :, 0], xt[:])
            ones_t = onep.tile([P, 1], f32)
            nc.vector.memset(ones_t[:], 1.0)

            for _rep in range(repeat):
                for ks, ke in groups:
                    if ke <= ks:
                        continue
                    hr0 = ks * rpc
                    hr1 = min(ke * rpc, rows_per_core)
                    dts = dtp.tile([P, gspan], f16, tag="dts")
                    for k in range(ks, ke):
                        rb = k * rpc
                        S = int(S_list[k])
                        epl = int(epl_list[k])
                        rpck = min(rpc, rows_per_core - rb)
                        sbase = int(sum(S_list[:k]))
                        mbase = (sbase + int(sum(epl_list[:k]))) // 16

                        mt = metap.tile([P, (S + epl) // 16], i16, tag="mt")
                        nc.sync.dma_start(
                            mt[:], meta[:, mbase : mbase + (S + epl) // 16]
                        )
                        vt = vtp.tile([P, S_max, 1], f32, tag="vt")
                        src_v = bass.AP(valsd, sbase, [[Ltot, NBUK], [0, 16], [1, S]])
                        nc.sync.dma_start(vt[:, 0:S, 0], src_v)
                        gt = gtp.tile([P, S_max, 1], f32, tag="gt")
                        nc.gpsimd.ap_gather(
                            out_ap=gt[:, 0:S, :],
                            in_ap=tab_t[:],
                            idxs_ap=mt[:, 0 : S // 16],
                            channels=P,
                            num_elems=BUK,
                            d=1,
                            num_idxs=S,
                        )
                        nc.vector.tensor_tensor(
                            out=gt[:, 0:S, 0],
                            in0=gt[:, 0:S, 0],
                            in1=vt[:, 0:S, 0],
                            op=mybir.AluOpType.mult,
                        )
                        nc.vector.tensor_tensor_scan(
                            out=vt[:, 0:S, 0],
                            data0=ones_t[:].to_broadcast([P, S]),
                            data1=gt[:, 0:S, 0],
                            initial=0.0,
                            op0=mybir.AluOpType.mult,
                            op1=mybir.AluOpType.add,
                        )
                        et = etp.tile([P, epl_max, 1], f32, tag="et")
                        nc.gpsimd.ap_gather(
                            out_ap=et[:, 0:epl, :],
                            in_ap=vt[:],
                            idxs_ap=mt[:, S // 16 : (S + epl) // 16],
                            channels=P,
                            num_elems=S_max,
                            d=1,
                            num_idxs=epl,
                        )
                        nc.vector.tensor_tensor(
                            out=dts[:, rb - hr0 : rb - hr0 + rpck],
                            in0=et[:, 1 : rpck + 1, 0],
                            in1=et[:, 0:rpck, 0],
                            op=mybir.AluOpType.subtract,
                        )
                    # fold 8 buckets: partitions 16c+j summed into j (j<16)
                    w = hr1 - hr0
                    for span in (64, 32, 16):
                        ft = fp.tile([64, gspan], f16, tag="ft")
                        nc.sync.dma_start(ft[0:span, 0:w], dts[span : 2 * span, 0:w])
                        nc.vector.tensor_tensor(
                            out=dts[0:span, 0:w],
                            in0=dts[0:span, 0:w],
                            in1=ft[0:span, 0:w],
                            op=mybir.AluOpType.add,
                        )
                    nc.sync.dma_start(y[:, hr0:hr1], dts[0:B, 0:w])
    nc.compile()
    return nc


def _full_prep(X, vals, rows, cols, rows_per_core, rpc, n_cores):
    n_chunks = -(-rows_per_core // rpc)
    bounds = np.searchsorted(rows, np.arange(n_cores + 1) * rows_per_core)
    preps = []
    for n in range(n_cores):
        k0, k1 = bounds[n], bounds[n + 1]
        preps.append(
            _prep_core(
                (rows[k0:k1] - n * rows_per_core).astype(np.int64),
                cols[k0:k1].astype(np.int64),
                vals[k0:k1],
                rows_per_core,
                rpc,
            )
        )
    need = np.stack([p["need"] for p in preps])  # [cores, buk, chunks]
    S_list = [int(_ceil_to(int(need[:, :, k].max()), 64)) for k in range(n_chunks)]
    epl_list = []
    for k in range(n_chunks):
        r0, r1 = k * rpc, min((k + 1) * rpc, rows_per_core)
        epl_list.append(int(_ceil_to(r1 - r0 + 1, 64)))
    assert max(S_list) <= 17000, f"chunk too big: {max(S_list)}"

    T = np.zeros((P, BUK), np.float32)
    for c in range(NBUK):
        T[16 * c : 16 * c + 8, :] = X[:, BUK * c : BUK * (c + 1)]

    in_maps = []
    for n in range(n_cores):
        lay = _layout_core(preps[n], S_list, epl_list, rows_per_core, rpc)
        in_maps.append({"xt": T, "meta": lay["meta"], "valsd": lay["valsd"]})
    return S_list, epl_list, in_maps


def kernel(X, vals, rows, cols):
    X = np.asarray(X, np.float32)
    vals = np.asarray(vals, np.float32)
    rows = np.asarray(rows, np.int64)
    cols = np.asarray(cols, np.int64)
    rows_per_core = N_ROWS // N_CORES

    rpc = RPC
    while True:
        try:
            S_list, epl_list, in_maps = _full_prep(
                X, vals, rows, cols, rows_per_core, rpc, N_CORES
            )
            break
        except AssertionError:
            rpc //= 2
            if rpc < 64:
                raise
    key = (tuple(S_list), tuple(epl_list), rpc, rows_per_core)
    if key not in _compiled:
        _compiled[key] = _build_nc(S_list, epl_list, rpc, rows_per_core)
    nc = _compiled[key]
    res = run_bass_kernel_spmd(nc, in_maps, core_ids=list(range(N_CORES)))
    Y = np.concatenate(
        [res.results[n]["y"].astype(np.float32) for n in range(N_CORES)], axis=1
    )
    return np.ascontiguousarray(Y, dtype=np.float32)


# revision 4
# speedup vs baseline: 11.0206x; 1.3183x over previous
"""Batched sparse projection Y[b,r] = sum_k vals[k]*X[b,cols[k]] for rows[k]==r.

Rewritten for this axon/TRN2 stack where per-core instructions execute
serially at ~25-50us fixed cost each (+ data slopes). Strategy: minimize
instruction count, maximize per-instruction work.

- 8 NeuronCores row-sharded (16384 rows each), nnz slice via searchsorted.
- Per core: nnz bucketed by col>>13 into 8 buckets = 8 GPSIMD Q7 cores.
  Table [128, 8192] f32: partition 16c+j (j<8) holds X[j, 8192c:8192(c+1)].
- Chunks of RPC=1024 rows (16 chunks). Per chunk only 7 instructions:
  1 meta DMA (idx+ends lists packed), 1 replicated vals DMA (stride-0 AP),
  1 ap_gather (num_idxs~16.5K), 1 in-place DVE mult, 1 DVE cumsum scan
  (output overwrites the vals buffer), 1 ends-extract ap_gather, 1 diff
  into f16 staging.
- Bucket merge: per 8192-row half, 3 log2 partition-fold steps
  (SBUF->SBUF shift DMA + DVE f16 add), then one f16 DMA out; host
  converts to f32.
"""

import numpy as np

import concourse.bass as bass
import concourse.mybir as mybir
import concourse.tile as tile
from concourse import bacc
from concourse.bass_utils import run_bass_kernel_spmd

B = 8
N_PIX = 65536
N_ROWS = 131072
N_CORES = 8
NBUK = 8
BUK = N_PIX // NBUK  # 8192
P = 128
RPC = 992  # rows per chunk

_compiled = {}


def _ceil_to(x, m):
    return -(-x // m) * m


def _prep_core(rows_l, cols_n, vals_n, rows_per_core, rpc):
    """Sort one core's nnz by (bucket, row); per-(bucket,chunk) counts."""
    buk = (cols_n >> 13).astype(np.int64)
    e = (cols_n & (BUK - 1)).astype(np.int16)
    key = buk * rows_per_core + rows_l.astype(np.int64)
    perm = np.argsort(key, kind="stable")
    skey = key[perm]
    cnt = np.bincount(key, minlength=NBUK * rows_per_core).reshape(NBUK, rows_per_core)
    n_chunks = -(-rows_per_core // rpc)
    need = np.zeros((NBUK, n_chunks), np.int64)
    for k in range(n_chunks):
        r0, r1 = k * rpc, min((k + 1) * rpc, rows_per_core)
        need[:, k] = cnt[:, r0:r1].sum(axis=1) + 1  # +1 zero slot
    return {"perm": perm, "skey": skey, "cnt": cnt, "e": e, "vals": vals_n,
            "need": need, "n_chunks": n_chunks}


def _layout_core(prep, S_list, epl_list, rows_per_core, rpc):
    """Build per-core meta (idx+ends wrapped int16) and valsd [8, Ltot]."""
    n_chunks = len(S_list)
    Ltot = int(sum(S_list))
    cnt = prep["cnt"]
    cbase = np.concatenate([[0], np.cumsum(S_list)]).astype(np.int64)

    skey, perm = prep["skey"], prep["perm"]
    c_sorted = skey // rows_per_core
    r_sorted = skey % rows_per_core
    chunk_id = r_sorted // rpc
    seg_key = c_sorted * n_chunks + chunk_id
    seg_cnt = np.bincount(seg_key, minlength=NBUK * n_chunks)
    seg_start = np.cumsum(seg_cnt) - seg_cnt
    rank = np.arange(skey.shape[0], dtype=np.int64) - seg_start[seg_key]
    pos = cbase[chunk_id] + 1 + rank  # +1 for the zero slot

    idx16 = np.zeros((NBUK, Ltot), np.int16)  # pad idx 0 (valid; val=0 kills it)
    valsd = np.zeros((NBUK, Ltot), np.float32)
    idx16[c_sorted, pos] = prep["e"][perm]
    valsd[c_sorted, pos] = prep["vals"][perm]

    # ends lists per chunk: [0, end(r0), end(r0+1), ...] padded with last
    ccnt = np.cumsum(cnt, axis=1)
    epx_parts = []
    for k in range(n_chunks):
        epl = epl_list[k]
        r0, r1 = k * rpc, min((k + 1) * rpc, rows_per_core)
        prev = ccnt[:, r0 - 1] if r0 > 0 else np.zeros(NBUK, np.int64)
        ends = ccnt[:, r0:r1] - prev[:, None]  # 1-based slot pos in chunk
        ep = np.zeros((NBUK, epl), np.int16)
        ep[:, 1 : 1 + (r1 - r0)] = ends.astype(np.int16)
        ep[:, 1 + (r1 - r0) :] = ends[:, -1:].astype(np.int16)
        epx_parts.append(ep)

    def wrap(a):  # [8, L] -> [128, L//16]; channel 16c+t <- a[c, 16s+t] at col s
        L = a.shape[1]
        return np.ascontiguousarray(
            a.reshape(NBUK, L // 16, 16).transpose(0, 2, 1)
        ).reshape(NBUK * 16, L // 16)

    # meta: per chunk [idxw_k | epw_k] concatenated along columns
    meta_parts = []
    for k in range(n_chunks):
        s0, s1 = int(cbase[k]), int(cbase[k + 1])
        meta_parts.append(wrap(idx16[:, s0:s1]))
        meta_parts.append(wrap(epx_parts[k]))
    meta = np.concatenate(meta_parts, axis=1)
    return {"meta": meta, "valsd": valsd}


def _build_nc(S_list, epl_list, rpc, rows_per_core, repeat=1):
    n_chunks = len(S_list)
    Ltot = int(sum(S_list))
    Mcols = int(sum(S_list[k] + epl_list[k] for k in range(n_chunks))) // 16
    nc = bacc.Bacc("TRN2", target_bir_lowering=False, debug=False)
    f32, f16, i16 = mybir.dt.float32, mybir.dt.float16, mybir.dt.int16

    xt = nc.dram_tensor("xt", [P, BUK], f32, kind="ExternalInput")
    meta = nc.dram_tensor("meta", [P, Mcols], i16, kind="ExternalInput")
    valsd = nc.dram_tensor("valsd", [NBUK, Ltot], f32, kind="ExternalInput")
    y = nc.dram_tensor("y", [B, rows_per_core], f16, kind="ExternalOutput")

    S_max = max(int(s) for s in S_list)
    epl_max = max(int(x) for x in epl_list)
    # two fold-groups split at a chunk boundary
    ksplit = -(-n_chunks // 2)
    groups = [(0, ksplit), (ksplit, n_chunks)]
    gspan = max(
        min(ke * rpc, rows_per_core) - ks * rpc for ks, ke in groups if ke > ks
    )

    with tile.TileContext(nc) as tc:
        with (
            tc.tile_pool(name="tabp", bufs=1) as tabp,
            tc.tile_pool(name="onep", bufs=1) as onep,
            tc.tile_pool(name="metap", bufs=1) as metap,
            tc.tile_pool(name="gtp", bufs=1) as gtp,
            tc.tile_pool(name="vtp", bufs=1) as vtp,
            tc.tile_pool(name="etp", bufs=1) as etp,
            tc.tile_pool(name="dtp", bufs=1) as dtp,
            tc.tile_pool(name="fp", bufs=1) as fp,
        ):
            tab_t = tabp.tile([P, BUK, 1], f32)
            nc.sync.dma_start(tab_t[:, :, 0], xt[:])
            ones_t = onep.tile([P, 1], f32)
            nc.vector.memset(ones_t[:], 1.0)

            for _rep in range(repeat):
                for ks, ke in groups:
                    if ke <= ks:
                        continue
                    hr0 = ks * rpc
                    hr1 = min(ke * rpc, rows_per_core)
                    dts = dtp.tile([P, gspan], f16, tag="dts")
                    for k in range(ks, ke):
                        rb = k * rpc
                        S = int(S_list[k])
                        epl = int(epl_list[k])
                        rpck = min(rpc, rows_per_core - rb)
                        sbase = int(sum(S_list[:k]))
                        mbase = (sbase + int(sum(epl_list[:k]))) // 16

                        mt = metap.tile([P, (S + epl) // 16], i16, tag="mt")
                        nc.sync.dma_start(
                            mt[:], meta[:, mbase : mbase + (S + epl) // 16]
                        )
                        vt = vtp.tile([P, S_max, 1], f32, tag="vt")
                        src_v = bass.AP(valsd, sbase, [[Ltot, NBUK], [0, 16], [1, S]])
                        nc.sync.dma_start(vt[:, 0:S, 0], src_v)
                        gt = gtp.tile([P, S_max, 1], f32, tag="gt")
                        nc.gpsimd.ap_gather(
                            out_ap=gt[:, 0:S, :],
                            in_ap=tab_t[:],
                            idxs_ap=mt[:, 0 : S // 16],
                            channels=P,
                            num_elems=BUK,
                            d=1,
                            num_idxs=S,
                        )
                        nc.vector.tensor_tensor(
                            out=gt[:, 0:S, 0],
                            in0=gt[:, 0:S, 0],
                            in1=vt[:, 0:S, 0],
                            op=mybir.AluOpType.mult,
                        )
                        nc.vector.tensor_tensor_scan(
                            out=vt[:, 0:S, 0],
                            data0=ones_t[:].to_broadcast([P, S]),
                            data1=gt[:, 0:S, 0],
                            initial=0.0,
                            op0=mybir.AluOpType.mult,
                            op1=mybir.AluOpType.add,
                        )
                        et = etp.tile([P, epl_max, 1], f32, tag="et")
                        nc.gpsimd.ap_gather(
                            out_ap=et[:, 0:epl, :],
                            in_ap=vt[:],
                            idxs_ap=mt[:, S // 16 : (S + epl) // 16],
                            channels=P,
                            num_elems=S_max,
                            d=1,
                            num_idxs=epl,
                        )
                        nc.vector.tensor_tensor(
                            out=dts[:, rb - hr0 : rb - hr0 + rpck],
                            in0=et[:, 1 : rpck + 1, 0],
                            in1=et[:, 0:rpck, 0],
                            op=mybir.AluOpType.subtract,
                        )
                    # fold 8 buckets: partitions 16c+j summed into j (j<16)
                    w = hr1 - hr0
                    for span in (64, 32, 16):
                        ft = fp.tile([64, gspan], f16, tag="ft")
                        nc.sync.dma_start(ft[0:span, 0:w], dts[span : 2 * span, 0:w])
                        nc.vector.tensor_tensor(
                            out=dts[0:span, 0:w],
                            in0=dts[0:span, 0:w],
                            in1=ft[0:span, 0:w],
                            op=mybir.AluOpType.add,
                        )
                    nc.sync.dma_start(y[:, hr0:hr1], dts[0:B, 0:w])
    nc.compile()
    return nc


def _full_prep(X, vals, rows, cols, rows_per_core, rpc, n_cores):
    n_chunks = -(-rows_per_core // rpc)
    bounds = np.searchsorted(rows, np.arange(n_cores + 1) * rows_per_core)
    preps = []
    for n in range(n_cores):
        k0, k1 = bounds[n], bounds[n + 1]
        preps.append(
            _prep_core(
                (rows[k0:k1] - n * rows_per_core).astype(np.int64),
                cols[k0:k1].astype(np.int64),
                vals[k0:k1],
                rows_per_core,
                rpc,
            )
        )
    need = np.stack([p["need"] for p in preps])  # [cores, buk, chunks]
    S_list = [int(_ceil_to(int(need[:, :, k].max()), 64)) for k in range(n_chunks)]
    epl_list = []
    for k in range(n_chunks):
        r0, r1 = k * rpc, min((k + 1) * rpc, rows_per_core)
        epl_list.append(int(_ceil_to(r1 - r0 + 1, 64)))
    assert max(S_list) <= 17000, f"chunk too big: {max(S_list)}"

    T = np.zeros((P, BUK), np.float32)
    for c in range(NBUK):
        T[16 * c : 16 * c + 8, :] = X[:, BUK * c : BUK * (c + 1)]

    in_maps = []
    for n in range(n_cores):
        lay = _layout_core(preps[n], S_list, epl_list, rows_per_core, rpc)
        in_maps.append({"xt": T, "meta": lay["meta"], "valsd": lay["valsd"]})
    return S_list, epl_list, in_maps


def kernel(X, vals, rows, cols):
    X = np.asarray(X, np.float32)
    vals = np.asarray(vals, np.float32)
    rows = np.asarray(rows, np.int64)
    cols = np.asarray(cols, np.int64)
    rows_per_core = N_ROWS // N_CORES

    rpc = RPC
    while True:
        try:
            S_list, epl_list, in_maps = _full_prep(
                X, vals, rows, cols, rows_per_core, rpc, N_CORES
            )
            key = (tuple(S_list), tuple(epl_list), rpc, rows_per_core)
            if key not in _compiled:
                _compiled[key] = _build_nc(S_list, epl_list, rpc, rows_per_core)
            break
        except (AssertionError, ValueError):
            # denser-than-expected chunks or SBUF alloc failure: shrink chunk
            rpc -= 128
            if rpc < 256:
                raise
    nc = _compiled[key]
    res = run_bass_kernel_spmd(nc, in_maps, core_ids=list(range(N_CORES)))
    Y = np.concatenate(
        [res.results[n]["y"].astype(np.float32) for n in range(N_CORES)], axis=1
    )
    return np.ascontiguousarray(Y, dtype=np.float32)
